# revision 1
# baseline (speedup 1.0000x reference)
"""AMPBlock0 (BigVGAN residual block) Trainium2 kernel.

Strategy: data-parallel over batch (B=8 -> 1 sample per NeuronCore).
Per core, everything stays in [C-on-partitions, T-free] layout, streamed
over T chunks:
  up1 (depthwise k=6, 2 phases)   -> PE diag-weight matmuls, PSUM
  snake1 (x + (1-cos(2a x))/2b)   -> ACT Sin with fp32 range reduction
  down1 (depthwise k=12 stride 2) -> PE diag matmuls on polyphase planes
  conv1 (dense 512x512 k=3)       -> PE bf16 matmuls
  up2/snake2/down2/conv2          -> same
  residual                        -> identity matmul into conv2 PSUM
"""

import math
import sys
import types

import numpy as np

MAGIC = 1.5 * 2**23  # fp32 round-to-nearest-int magic constant

B, C, T = 8, 512, 8192
NCC = C // 128  # channel chunks of 128 partitions
TOUT = T - 4

# tap->engine assignment tuning: list of (stage, tap) moved from PE to DVE
# stage in {"up1","down1","up2","down2"}; taps 0..11
TUNE = dict(dve_up=(), f_gp=False, pre_eng="split", sym=False, tc=1000,
            dve_down=(), t_eng="dve", s_eng="dve", r_eng="dve",
            res_eng="pe", cast_eng="dve", evac_bf=False)


def _install_axon_shim():
    """antenv.axon_hooks is missing in this image; recreate it so
    run_bass_kernel_spmd(trace=True) works. Harmless if tracing unused."""
    if "antenv.axon_hooks" in sys.modules:
        return
    try:
        import antenv

        mod = types.ModuleType("antenv.axon_hooks")
        _hook = [None]
        mod.set_axon_ntff_profile_hook = lambda h: _hook.__setitem__(0, h)
        mod.get_axon_ntff_profile_hook = lambda: _hook[0]
        sys.modules["antenv.axon_hooks"] = mod
        antenv.axon_hooks = mod
        from trn_agent_boot.trn_boot import _ntff_profile_via_ctypes

        mod.set_axon_ntff_profile_hook(
            _ntff_profile_via_ctypes("/opt/axon/libaxon_pjrt.so")
        )
    except Exception:
        pass


# ---------------------------------------------------------------------------
# Host-side weight preprocessing
# ---------------------------------------------------------------------------


def _prep_host(x, alpha1, beta1, alpha2, beta2, v1, g1, b1, v2, g2, b2,
               up_filt, down_filt):
    import ml_dtypes

    bf = ml_dtypes.bfloat16
    f32 = np.float32

    def wn(v, g):
        norm = np.sqrt(np.sum(v * v, axis=(1, 2), keepdims=True))
        return (g[:, None, None] * v / norm).astype(f32)

    w1 = wn(np.asarray(v1, f32), np.asarray(g1, f32))
    w2 = wn(np.asarray(v2, f32), np.asarray(g2, f32))
    upw = np.asarray(up_filt, f32)[:, 0, :]      # [2C, 6]
    dww = np.asarray(down_filt, f32)[:, 0, :]    # [C, 12]

    # weight blocks, each [128 rows, 128 cols], packed side by side in one
    # [128, nblk*128] tensor. Order:
    #   w1 lhsT blocks (k, ciq, coq)            : 3*4*4 = 48
    #   w2 lhsT blocks (k, ciq, coq)            : 48
    #   up diag blocks (p, j, q)                : 2*6*4 = 48
    #   down diag blocks (j, q)                 : 12*4 = 48
    #   identity                                : 1
    blocks = []
    for w in (w1, w2):
        for k in range(3):
            for ciq in range(NCC):
                for coq in range(NCC):
                    blk = w[coq * 128:(coq + 1) * 128,
                            ciq * 128:(ciq + 1) * 128, k].T  # [ci, co]
                    blocks.append(np.ascontiguousarray(blk))
    for p in range(2):
        for j in range(6):
            for q in range(NCC):
                d = upw[2 * np.arange(q * 128, (q + 1) * 128) + p, j]
                blocks.append(np.diag(d).astype(f32))
    for j in range(12):
        for q in range(NCC):
            d = dww[q * 128:(q + 1) * 128, j]
            blocks.append(np.diag(d).astype(f32))
    blocks.append(np.eye(128, dtype=f32))
    wt = np.concatenate(blocks, axis=1).astype(bf)  # [128, nblk*128]

    # per-channel vectors [128, NV*4]: col v*4+q = vec v for channel chunk q
    a1 = np.exp(np.asarray(alpha1, f32))
    a2 = np.exp(np.asarray(alpha2, f32))
    r1 = 1.0 / (2.0 * np.exp(np.asarray(beta1, f32)) + 1e-9)
    r2 = 1.0 / (2.0 * np.exp(np.asarray(beta2, f32)) + 1e-9)
    vec_list = [a1 / math.pi, r1, -r1, np.asarray(b1, f32),
                a2 / math.pi, r2, -r2, np.asarray(b2, f32),
                0.25 - a1 * r1 / math.pi, 0.25 - a2 * r2 / math.pi]
    NV = len(vec_list)
    vecs = np.zeros((128, NV * NCC), dtype=f32)
    for v, val in enumerate(vec_list):
        for q in range(NCC):
            vecs[:, v * NCC + q] = val[q * 128:(q + 1) * 128]

    # up filter per-partition scalars for DVE taps: [128, 2*6*4]
    upv = np.zeros((128, 2 * 6 * NCC), dtype=f32)
    for p in range(2):
        for j in range(6):
            for q in range(NCC):
                upv[:, (p * 6 + j) * NCC + q] = \
                    upw[2 * np.arange(q * 128, (q + 1) * 128) + p, j]
    dwv = np.zeros((128, 12 * NCC), dtype=f32)
    for j in range(12):
        for q in range(NCC):
            dwv[:, j * NCC + q] = dww[q * 128:(q + 1) * 128, j]

    xa = np.asarray(x, f32)
    x4 = np.ascontiguousarray(
        xa.reshape(xa.shape[0], NCC, 128, xa.shape[2]))  # [B,4,128,T]
    return x4, wt, vecs, upv, dwv


# block index helpers (must match _prep_host packing order)
def _blk_w(conv, k, ciq, coq):
    base = 0 if conv == 1 else 48
    return base + ((k * NCC) + ciq) * NCC + coq


def _blk_up(p, j, q):
    return 96 + (p * 6 + j) * NCC + q


def _blk_down(j, q):
    return 144 + j * NCC + q


_BLK_ID = 192
_NBLK = 193


# ---------------------------------------------------------------------------
# Device kernel builder
# ---------------------------------------------------------------------------


def build_kernel(T_=T, Tc=1000, sym_down=False, dve_up=(),
                 f_gp=False, pre_eng="vector", dve_down=(),
                 t_eng="dve", s_eng="dve", r_eng="dve",
                 res_eng="pe", cast_eng="dve", evac_bf=False,
                 stage_bufs=1, act_tags=False):
    import concourse.bacc as bacc
    import concourse.tile as tile
    from concourse import mybir

    bf16 = mybir.dt.bfloat16
    f32 = mybir.dt.float32
    AF = mybir.ActivationFunctionType
    ALU = mybir.AluOpType
    dve_up = set(dve_up)
    dve_down = tuple(dve_down)

    L1 = T_ - 2   # act1/conv1 length
    L2 = T_ - 4   # act2/conv2/out length
    nchunk = (L2 + Tc - 1) // Tc

    # stage paddings (left, right) relative to out chunk [t0, t0+Tc)
    PAD = dict(x=(10, 20), y1=(9, 15), z1=(6, 11), xt=(5, 10),
               y2=(4, 5), z2=(1, 1), out=(0, 0))
    # stage global validity
    VAL = dict(x=T_, y1=L1, z1=L1, xt=L1, y2=L2, z2=L2, out=L2)

    nc = bacc.Bacc()
    xin = nc.dram_tensor("x", [NCC, 128, T_], f32, kind="ExternalInput")
    wt = nc.dram_tensor("wt", [128, _NBLK * 128], bf16, kind="ExternalInput")
    vecs = nc.dram_tensor("vecs", [128, 10 * NCC], f32, kind="ExternalInput")
    upv = nc.dram_tensor("upv", [128, 12 * NCC], f32, kind="ExternalInput")
    dwv = nc.dram_tensor("dwv", [128, 12 * NCC], f32, kind="ExternalInput")
    out = nc.dram_tensor("out", [NCC, 128, L2], f32, kind="ExternalOutput")

    def rng_of(stage, t0, tcw):
        """global [lo, hi) actually computed for this stage tile, plus
        tile origin O (column 0 of the tile == global O)."""
        pl, pr = PAD[stage]
        O = t0 - pl
        lo = max(0, O)
        hi = min(VAL[stage], t0 + tcw + pr)
        return O, lo, hi

    with tile.TileContext(nc) as tc:
        pools = {}

        def pool(name, bufs):
            if name not in pools:
                pools[name] = tc.alloc_tile_pool(name=name, bufs=bufs)
            return pools[name]

        consts = pool("consts", 1)
        wt_sb = consts.tile([128, _NBLK * 128], bf16)
        nc.sync.dma_start(wt_sb[:], wt[:])
        vec_sb = consts.tile([128, 10 * NCC], f32)
        nc.sync.dma_start(vec_sb[:], vecs[:])
        upv_sb = consts.tile([128, 12 * NCC], f32)
        nc.sync.dma_start(upv_sb[:], upv[:])
        dwv_sb = consts.tile([128, 12 * NCC], f32)
        nc.sync.dma_start(dwv_sb[:], dwv[:])

        def WB(i):
            return wt_sb[:, i * 128:(i + 1) * 128]

        def VEC(v, q):
            return vec_sb[:, v * NCC + q:v * NCC + q + 1]

        # psum pools: main [128,1024] (2 banks) x3, tail [128,512] x2
        ps_main = tc.alloc_tile_pool(name="ps_main", bufs=3, space="PSUM")
        ps_tail = tc.alloc_tile_pool(name="ps_tail", bufs=2, space="PSUM")

        def psum_pieces(width):
            """yield (psum_tile, col0, w) covering [0, width)"""
            pieces = []
            off = 0
            while off < width:
                w = min(1024, width - off)
                if w > 512:
                    tl = ps_main.tile([128, 1024], f32, tag="psm")
                else:
                    tl = ps_tail.tile([128, 512], f32, tag="pst")
                pieces.append((tl, off, w))
                off += w
            return pieces

        def mm_into(pieces, blk_lhsT, rhs_tile, rhs_col0, start, stop):
            """accumulate lhsT.T @ rhs into the psum pieces; rhs columns are
            read starting at rhs_col0 (aligned with piece col0)."""
            for tl, off, w in pieces:
                sub = 0
                while sub < w:
                    n = min(512, w - sub)
                    nc.tensor.matmul(
                        tl[:, sub:sub + n],
                        blk_lhsT,
                        rhs_tile[:, rhs_col0 + off + sub:rhs_col0 + off + sub + n],
                        start=start, stop=stop)
                    sub += n

        # ---- per chunk ----
        for ci in range(nchunk):
            t0 = ci * Tc
            tcw = min(Tc, L2 - t0)

            # 1) x in: DMA fp32 staging -> cast bf16
            Ox, xlo, xhi = rng_of("x", t0, tcw)
            Wx = PAD["x"][0] + tcw + PAD["x"][1]
            xbf = {}
            for q in range(NCC):
                st = pool("xstage", 2).tile([128, Wx], f32, tag="xs")
                nc.sync.dma_start(st[:, xlo - Ox:xhi - Ox],
                                  xin[q, :, xlo:xhi])
                xb = pool("xbf", stage_bufs).tile([128, Wx], bf16, tag=f"xb{q}")
                if xlo - Ox > 0:
                    nc.gpsimd.memset(xb[:, 0:xlo - Ox], 0.0)
                if Wx - (xhi - Ox) > 0:
                    nc.gpsimd.memset(xb[:, xhi - Ox:Wx], 0.0)
                if cast_eng == "act":
                    nc.scalar.copy(xb[:, xlo - Ox:xhi - Ox],
                                   st[:, xlo - Ox:xhi - Ox])
                elif cast_eng == "gp":
                    nc.gpsimd.tensor_copy(xb[:, xlo - Ox:xhi - Ox],
                                          st[:, xlo - Ox:xhi - Ox])
                else:
                    nc.vector.tensor_copy(xb[:, xlo - Ox:xhi - Ox],
                                          st[:, xlo - Ox:xhi - Ox])
                xbf[q] = xb

            def act_block(src, src_stage, ystage, zstage, vbase, c2idx, tag):
                """activation1d: src tiles (bf16, per cc) -> z tiles (bf16).
                vbase: 0 for act1 (a1,r1,-r1), 4 for act2."""
                Os, slo, shi = rng_of(src_stage, t0, tcw)
                Oy, ylo, yhi = rng_of(ystage, t0, tcw)
                Wy = PAD[ystage][0] + tcw + PAD[ystage][1]
                yw = yhi - ylo
                ydt = bf16 if evac_bf else f32
                # y' planes (both phases side by side) + s (bf16)
                yp = {}
                sp = {}
                for q in range(NCC):
                    # --- upsample phase p: y_p[g] = sum_j upw*src[g+j-1]
                    ypl = pool("yp", stage_bufs).tile([128, 2, Wy], ydt,
                                             tag=f"yp{q}{tag if act_tags else ''}")
                    spl = pool("sp", stage_bufs).tile([128, 2, Wy], bf16,
                                             tag=f"sp{q}{tag if act_tags else ''}")
                    for p in range(2):
                        if ylo - Oy > 0:
                            nc.gpsimd.memset(spl[:, p, 0:ylo - Oy], 0.0)
                        if Wy - (yhi - Oy) > 0:
                            nc.gpsimd.memset(spl[:, p, yhi - Oy:Wy], 0.0)
                        pieces = psum_pieces(yw)
                        # rhs read offset for tap j: global ylo+j-1 ->
                        # local (ylo + j - 1) - Os
                        pe_taps = [j for j in range(6) if (p, j) not in dve_up]
                        assert pe_taps, "need >=1 PE tap per phase"
                        for i, j in enumerate(pe_taps):
                            mm_into(pieces, WB(_blk_up(p, j, q)), src[q],
                                    (ylo + j - 1) - Os,
                                    start=(i == 0), stop=(i == len(pe_taps) - 1))
                        # evac psum -> y' with bias +r
                        for tl, off, w in pieces:
                            nc.scalar.activation(
                                ypl[:, p, ylo - Oy + off:ylo - Oy + off + w],
                                tl[:, 0:w], AF.Identity,
                                bias=VEC(vbase + 1, q), scale=1.0)
                        # remaining taps: independent bf16 partial plane on
                        # DVE (ts 4x + tt 2x), merged once after the evac
                        dj = [j for j in range(6) if (p, j) in dve_up]
                        if dj:
                            yvt = pool("yv", 1).tile([128, Wy], bf16,
                                                     tag=f"yv{q}p{p}")
                            tmv = None
                            if len(dj) > 1:
                                tmv = pool("yv", 1).tile([128, Wy], bf16,
                                                         tag=f"yvt{q}")
                            for i, j in enumerate(dj):
                                sc = upv_sb[:, (p * 6 + j) * NCC + q:
                                            (p * 6 + j) * NCC + q + 1]
                                srcv = src[q][:, (ylo + j - 1) - Os:
                                              (ylo + j - 1) - Os + yw]
                                if i == 0:
                                    nc.vector.tensor_scalar_mul(
                                        yvt[:, 0:yw], srcv, sc)
                                else:
                                    nc.vector.tensor_scalar_mul(
                                        tmv[:, 0:yw], srcv, sc)
                                    nc.vector.tensor_add(
                                        yvt[:, 0:yw], yvt[:, 0:yw],
                                        tmv[:, 0:yw])
                            yv_ = ypl[:, p, ylo - Oy:yhi - Oy]
                            nc.vector.tensor_tensor(
                                yv_, yv_, yvt[:, 0:yw], op=ALU.add)
                    yp[q] = ypl
                    sp[q] = spl

                # --- snake on both phases at once: [128, 2*Wy] view
                for q in range(NCC):
                    ypl, spl = yp[q], sp[q]
                    tt = pool("tred", 2).tile([128, 2, Wy], f32, tag="t")
                    rm = pool("tred", 2).tile([128, 2, Wy], f32, tag="rm")
                    uu = pool("usin", 2).tile([128, 2, Wy], bf16, tag="u")
                    a_ = ypl[:, :, ylo - Oy:yhi - Oy]
                    t_ = tt[:, :, ylo - Oy:yhi - Oy]
                    r_ = rm[:, :, ylo - Oy:yhi - Oy]
                    u_ = uu[:, :, ylo - Oy:yhi - Oy]
                    s_ = spl[:, :, ylo - Oy:yhi - Oy]
                    # t = y'*(a/pi) + (0.25 - (a/pi)*r) ... y' = y + r
                    # so psi/2pi = (a/pi)*(y' - r) + 0.25
                    # scalar2 = 0.25 - a*r/pi  (needs per-channel const):
                    # use two-op ts with vector scalars: (y' mult a/pi) add c2
                    if t_eng == "act":
                        nc.scalar.activation(
                            t_, a_, AF.Identity,
                            bias=VEC(c2idx, q), scale=VEC(vbase + 0, q))
                    else:
                        nc.vector.tensor_scalar(
                            t_, a_, VEC(vbase + 0, q), VEC(c2idx, q),
                            op0=ALU.mult, op1=ALU.add)
                    reng = nc.gpsimd if r_eng == "gp" else nc.vector
                    reng.tensor_scalar(
                        r_, t_, MAGIC, MAGIC, op0=ALU.add, op1=ALU.subtract)
                    feng = nc.gpsimd if f_gp else nc.vector
                    feng.tensor_tensor(r_, t_, r_, op=ALU.subtract)
                    nc.scalar.activation(u_, r_, AF.Sin, bias=0.0,
                                         scale=float(2 * math.pi))
                    # s = y' - r*u
                    seng = nc.gpsimd if s_eng == "gp" else nc.vector
                    seng.scalar_tensor_tensor(
                        s_, u_, VEC(vbase + 2, q), a_,
                        op0=ALU.mult, op1=ALU.add)

                # --- downsample: z[g] = sum_m dwE[m]*s1[g+m-3] + dwO[m]*s0[g+m-2]
                Oz, zlo, zhi = rng_of(zstage, t0, tcw)
                Wz = PAD[zstage][0] + tcw + PAD[zstage][1]
                zw = zhi - zlo
                z = {}
                for q in range(NCC):
                    spl = sp[q]
                    zt = pool("z", stage_bufs).tile([128, Wz], bf16,
                                           tag=f"z{q}{tag if act_tags else ''}")
                    if zlo - Oz > 0:
                        nc.gpsimd.memset(zt[:, 0:zlo - Oz], 0.0)
                    if Wz - (zhi - Oz) > 0:
                        nc.gpsimd.memset(zt[:, zhi - Oz:Wz], 0.0)
                    pieces = psum_pieces(zw)
                    # taps offloaded to DVE: independent partial-sum plane zv,
                    # combined with the PSUM evac at the end. Taps chosen with
                    # even tile-local read offsets (interior) for ts 4x mode.
                    if dve_down:
                        zv = pool("zv", 1).tile([128, Wz], bf16, tag=f"zv{q}")
                        tmpv = pool("zv", 1).tile([128, Wz], bf16,
                                                  tag=f"zvt{q}")
                        for i, j in enumerate(dve_down):
                            p = 1 if j % 2 == 0 else 0
                            m = j // 2
                            goff = m - 3 if j % 2 == 0 else m - 2
                            srcv = spl[:, p, (zlo + goff) - Oy:
                                       (zlo + goff) - Oy + zw]
                            sc = dwv_sb[:, j * NCC + q:j * NCC + q + 1]
                            if i == 0:
                                nc.vector.tensor_scalar_mul(
                                    zv[:, 0:zw], srcv, sc)
                            else:
                                nc.vector.tensor_scalar_mul(
                                    tmpv[:, 0:zw], srcv, sc)
                                nc.vector.tensor_add(
                                    zv[:, 0:zw], zv[:, 0:zw], tmpv[:, 0:zw])
                    if sym_down:
                        # dw[2m] == dw[2(5-m)+1]:
                        # z = sum_m dw[2m]*(s1[g+m-3] + s0[g+3-m])
                        for m in range(6):
                            d = m - 3
                            if pre_eng == "split":
                                peng = nc.vector if d % 2 != 0 else nc.gpsimd
                            elif pre_eng == "gpsimd":
                                peng = nc.gpsimd
                            else:
                                peng = nc.vector
                            pre = pool("pre", 2).tile([128, Wz], bf16,
                                                      tag=f"pre{q}")
                            peng.tensor_tensor(
                                pre[:, 0:zw],
                                spl[:, 1, (zlo + d) - Oy:(zlo + d) - Oy + zw],
                                spl[:, 0, (zlo - d) - Oy:(zlo - d) - Oy + zw],
                                op=ALU.add)
                            mm_into(pieces, WB(_blk_down(2 * m, q)),
                                    pre, 0,
                                    start=(m == 0), stop=(m == 5))
                    else:
                        pe_taps = [j for j in range(12) if j not in dve_down]
                        for i, j in enumerate(pe_taps):
                            p = 1 if j % 2 == 0 else 0
                            m = j // 2
                            # even j -> phase1 g+m-3 ; odd j -> phase0 g+m-2
                            goff = m - 3 if j % 2 == 0 else m - 2
                            mm_into(pieces, WB(_blk_down(j, q)),
                                    spl[:, p, :], (zlo + goff) - Oy,
                                    start=(i == 0),
                                    stop=(i == len(pe_taps) - 1))
                    for tl, off, w in pieces:
                        nc.scalar.activation(
                            zt[:, zlo - Oz + off:zlo - Oz + off + w],
                            tl[:, 0:w], AF.Identity, bias=0.0, scale=1.0)
                    if dve_down:
                        nc.vector.tensor_add(
                            zt[:, zlo - Oz:zlo - Oz + zw],
                            zt[:, zlo - Oz:zlo - Oz + zw], zv[:, 0:zw])
                    z[q] = zt
                return z, Oz

            # ---------------- act1
            z1, Oz1 = act_block(xbf, "x", "y1", "z1", 0, 8, "1")

            # ---------------- conv1: xt[o,g] = sum_k,c w1*z1[c,g+k-1] + b1
            Ox1, xt_lo, xt_hi = rng_of("xt", t0, tcw)
            Wxt = PAD["xt"][0] + tcw + PAD["xt"][1]
            xtw = xt_hi - xt_lo
            xt = {}
            for coq in range(NCC):
                xtt = pool("xt", stage_bufs).tile([128, Wxt], bf16, tag=f"xt{coq}")
                if xt_lo - Ox1 > 0:
                    nc.gpsimd.memset(xtt[:, 0:xt_lo - Ox1], 0.0)
                if Wxt - (xt_hi - Ox1) > 0:
                    nc.gpsimd.memset(xtt[:, xt_hi - Ox1:Wxt], 0.0)
                pieces = psum_pieces(xtw)
                first = True
                for k in range(3):
                    for ciq in range(NCC):
                        mm_into(pieces, WB(_blk_w(1, k, ciq, coq)), z1[ciq],
                                (xt_lo + k - 1) - Oz1,
                                start=first, stop=(k == 2 and ciq == NCC - 1))
                        first = False
                for tl, off, w in pieces:
                    nc.scalar.activation(
                        xtt[:, xt_lo - Ox1 + off:xt_lo - Ox1 + off + w],
                        tl[:, 0:w], AF.Identity, bias=VEC(3, coq), scale=1.0)
                xt[coq] = xtt

            # ---------------- act2
            z2, Oz2 = act_block(xt, "xt", "y2", "z2", 4, 9, "2")

            # ---------------- conv2 + residual + b2 -> out
            for coq in range(NCC):
                ow = tcw
                pieces = psum_pieces(ow)
                first = True
                last = res_eng != "pe"
                for k in range(3):
                    for ciq in range(NCC):
                        mm_into(pieces, WB(_blk_w(2, k, ciq, coq)), z2[ciq],
                                (t0 + k - 1) - Oz2, start=first,
                                stop=(last and k == 2 and ciq == NCC - 1))
                        first = False
                if res_eng == "pe":
                    # residual: += I @ x (same channel chunk)
                    mm_into(pieces, WB(_BLK_ID), xbf[coq],
                            t0 - Ox, start=False, stop=True)
                ot = pool("outst", 2).tile([128, Tc], f32, tag="o")
                for tl, off, w in pieces:
                    nc.scalar.activation(
                        ot[:, off:off + w], tl[:, 0:w], AF.Identity,
                        bias=VEC(7, coq), scale=1.0)
                if res_eng != "pe":
                    reseng = nc.gpsimd if res_eng == "gp" else nc.vector
                    reseng.tensor_tensor(
                        ot[:, 0:ow], ot[:, 0:ow],
                        xbf[coq][:, t0 - Ox:t0 - Ox + ow], op=ALU.add)
                nc.sync.dma_start(out[coq, :, t0:t0 + ow], ot[:, 0:ow])

        ps_tail.release()
        ps_main.release()
        for p in reversed(list(pools.values())):
            p.release()

    nc.finalize()
    return nc


# ---------------------------------------------------------------------------
# Public entry point
# ---------------------------------------------------------------------------

_CACHED = {}


def kernel(**inputs):
    _install_axon_shim()
    from concourse import bass_utils

    x4, wt, vecs, upv, dwv = _prep_host(**inputs)

    dww = np.asarray(inputs["down_filt"], np.float32)[:, 0, :]
    sym = bool(np.array_equal(dww, dww[:, ::-1])) and TUNE["sym"]
    key = ("nc", sym)
    if key not in _CACHED:
        _CACHED[key] = build_kernel(Tc=TUNE["tc"], sym_down=sym,
                                    dve_up=TUNE["dve_up"], f_gp=TUNE["f_gp"],
                                    pre_eng=TUNE["pre_eng"],
                                    dve_down=TUNE["dve_down"],
                                    t_eng=TUNE["t_eng"], s_eng=TUNE["s_eng"],
                                    r_eng=TUNE["r_eng"],
                                    res_eng=TUNE["res_eng"],
                                    cast_eng=TUNE["cast_eng"],
                                    evac_bf=TUNE["evac_bf"])
    nc = _CACHED[key]

    in_maps = []
    for b in range(B):
        in_maps.append(dict(x=np.ascontiguousarray(x4[b]), wt=wt,
                            vecs=vecs, upv=upv, dwv=dwv))
    res = bass_utils.run_bass_kernel_spmd(nc, in_maps, core_ids=list(range(B)))
    outs = [res.results[b]["out"].reshape(C, TOUT) for b in range(B)]
    return np.stack(outs, axis=0).astype(np.float32)

